# revision 5
# baseline (speedup 1.0000x reference)
"""TRN2 Bass kernel for nn_Attention_188978561266.

Reference computation (b=4, s=1024, d=1024, 16 heads x 64):
    qkv = x @ Wqkv ; split q,k,v
    q = q / (sqrt(mean(q^2 over ALL elements)) + eps) * scale_q   (global scalar RMS)
    k = k / (sqrt(mean(k^2 over ALL elements)) + eps) * scale_k
    attn = softmax(q @ k^T)  (no 1/sqrt(d_head), no mask)
    out = (attn @ v) @ Wo

Sharding: 8 cores = (batch b in 0..3) x (head-half in 0..1). Each core computes
qkv for its batch restricted to its 8 heads (tensor-parallel QKV columns),
full attention for those heads, and a partial (transposed) output projection.
Host sums the two partial outputs per batch and transposes. The global RMS
needs a cross-core AllReduce of the (sum q^2, sum k^2) scalars.

v2 design notes (measured on HW via neuron-profile traces):
  - Phase C is ACT-exp-bound: 64 exp tiles of [128,1024] at ~1.15us each.
    v1 ran PE and ACT in per-tile lockstep; the PE's micro-idles kept the
    HAM clock gate at K=4/8 (1.2GHz) where the PE cannot feed the exp
    stream, locking a ~1.95us/tile slow state. v2 decouples them:
      * During the collective wait the PE pre-computes NSTG S tiles whose
        psums the DVE copies into an SBUF stage ring; the ring is an
        ~11us exp-credit buffer absorbing any PE clock transient.
      * Steady state exps read S psums directly (DVE psum-copies are 1x
        mode and cannot keep up as an intermediary).
      * The V projection (A2) is interleaved into the staging window as
        real PE filler so the PE stream has no idle gaps there.
      * AV matmuls trail the exp stream with a lag that starts at NSTG
        and shrinks to 2, so the tail drain is minimal.
  - One ACT table set for the whole kernel (natural_log_exp_and_others):
    the global scale is c = exp(-0.5*ln(mq*mk)) (no Sqrt table), Square
    and Copy are fillers present in every set, and a dummy Ln at t0 pins
    the set. No table switch ever lands on the critical path.
  - The stats partition-reduction for the critical k half is a PE
    ones-matmul (f32): out[128,1] = ones128.T @ qk2 gives every partition
    the cross-partition sum in ~0.2us (vs ~1.3us gpsimd ucode). The q
    half (off critical path, PE busy) stays on gpsimd.
  - Collectives carry [128,1] replicated payloads so the AllReduce result
    DMAs straight into per-partition SBUF: no partition_broadcast after
    the collective. c_bc = exp(-0.5 ln(gq*gk/COUNT^2)) is computed
    per-partition in ~0.5us (DVE mult + 2 tiny ACT ops).
  - The warm dummy collective (absorbs ~35us CC cold start) and the split
    q-half/k-half collectives (q absorbs cross-core skew so k runs warm)
    are retained from v1.
  - Output projection at the tail, Wo-stationary, two 4-dout waves; zT
    chunk 3 is head-swapped so the last head takes the no-DMA path;
    psum->SBUF copies alternate DVE / ACT (ACT is free at the tail).
  - PSUM budget: 2x psp + 2x pav tiles of [128,2,512] = 8 banks.
"""

import os as _os
import sys

sys.path.insert(0, "/opt/trn_rl_repo")

import ml_dtypes
import numpy as np

import concourse.bacc as bacc
import concourse.bass as bass
import concourse.mybir as mybir
from concourse import bass_isa, library_config, tile
from concourse.bass_utils import run_bass_kernel_spmd

F32 = mybir.dt.float32
F32R = mybir.dt.float32r
BF16 = mybir.dt.bfloat16
AF = mybir.ActivationFunctionType
ALU = mybir.AluOpType
AX = mybir.AxisListType

NPBF = ml_dtypes.bfloat16

P = 128
D = 1024
S = 1024
N_HEAD = 16
DH = 64
NHL = 8          # heads per core
DC = 8           # d contraction chunks of 128
EPS = 1e-6
COUNT = 4 * 1024 * 1024   # elements of the full q (or k) tensor
N_CORES = 8
REPLICAS = [list(range(N_CORES))]

NSTG = int(_os.environ.get("KN_NSTG", "9"))   # staged S tiles (exp credit)

_CACHE = {}


def _build():
    nc = bacc.Bacc("TRN2", target_bir_lowering=False, debug=False, num_devices=N_CORES)

    xt = nc.dram_tensor("xt", [P, DC, S], BF16, kind="ExternalInput")
    wqk = nc.dram_tensor("wqk", [P, 8, DC, P], BF16, kind="ExternalInput")
    wv = nc.dram_tensor("wv", [P, DC, NHL * DH], BF16, kind="ExternalInput")
    wo = nc.dram_tensor("wo", [P, 4, D], BF16, kind="ExternalInput")
    qscale = nc.dram_tensor("qscale", [P, 4], F32, kind="ExternalInput")
    zpartT = nc.dram_tensor("zpartT", [D, S], F32, kind="ExternalOutput")

    with tile.TileContext(nc) as tc:
        with (
            tc.tile_pool(name="big", bufs=1) as big,
            tc.tile_pool(name="ep", bufs=3) as ep,
            tc.tile_pool(name="scr", bufs=2) as scrp,
            tc.tile_pool(name="ob", bufs=2) as obp,
            tc.tile_pool(name="small", bufs=2) as smallp,
            tc.tile_pool(name="stats", bufs=1) as stp,
            tc.tile_pool(name="ps", bufs=2, space="PSUM") as psp,
            tc.tile_pool(name="pav", bufs=2, space="PSUM") as pav,
            tc.tile_pool(name="dram", bufs=1, space="DRAM") as dramp,
        ):
            # ---- persistent SBUF tensors ----
            xT = big.tile([P, DC, S], BF16, tag="xT")
            Wqk_sb = big.tile([P, 8, DC, P], BF16, tag="Wqk")
            QT = big.tile([P, 4, S], BF16, tag="QT")
            KT = big.tile([P, 4, S], BF16, tag="KT")
            Vt = big.tile([P, 8, NHL, DH + 1], BF16, tag="Vt")
            zT = big.tile([P, 4, S], BF16, tag="zT")
            Wo_sb = big.tile([P, 4, D], BF16, tag="Wo")
            Wv_sb = big.tile([P, DC, NHL * DH], BF16, tag="Wv")

            Sstage = big.tile([P, NSTG, S], F32, tag="Sstage")
            ones128 = big.tile([P, P], F32, tag="ones128")
            qs_sb = stp.tile([P, 4], F32, tag="qs")
            sq_acc = stp.tile([P, 8], F32, tag="sqacc")
            qk2 = stp.tile([P, 2], F32, tag="qk2")
            qred = stp.tile([P, 2], F32, tag="qred")
            gsum_bc = stp.tile([P, 2], F32, tag="gsum")
            prod = stp.tile([P, 1], F32, tag="prod")
            lnp = stp.tile([P, 1], F32, tag="lnp")
            c_bc = stp.tile([P, 1], F32, tag="cbc")

            dummy = stp.tile([1, 2], F32, tag="dummy")
            ones_row = stp.tile([1, P], F32, tag="ones_row")
            ones_blk = stp.tile([P, 8, NHL, 1], F32, tag="ones_blk")

            # warm collective first: starts the CC firmware warmup / skew
            # barrier as early as possible (it does not need the library).
            cc_warm_in = dramp.tile([2, 1], F32, tag="ccwi")
            cc_warm_out = dramp.tile([2, 1], F32, tag="ccwo", addr_space="Shared")
            cc_ins = [
                dramp.tile([P, 1], F32, tag=f"cci{h}", name=f"cc_in{h}")
                for h in range(2)
            ]
            cc_outs = [
                dramp.tile(
                    [P, 1], F32, tag=f"cco{h}", addr_space="Shared",
                    name=f"cc_out{h}",
                )
                for h in range(2)
            ]
            if _os.environ.get("KN_WARMCC", "1") == "1":
                nc.gpsimd.collective_compute(
                    "AllReduce",
                    ALU.add,
                    replica_groups=REPLICAS,
                    ins=[cc_warm_in[:]],
                    outs=[cc_warm_out[:]],
                )
            nc.gpsimd.load_library(library_config.attn)

            # ---- input DMAs + consts; wqk cts 0-3 then x chunks in arrival
            # order on two queues so the dc-outer phase A starts earliest.
            for q in range(4):
                nc.sync.dma_start(
                    xT[:, 2 * q : 2 * q + 2, :], xt[:, 2 * q : 2 * q + 2, :]
                )
                nc.scalar.dma_start(
                    Wqk_sb[:, 2 * q : 2 * q + 2, :, :],
                    wqk[:, 2 * q : 2 * q + 2, :, :],
                )
            nc.sync.dma_start(qs_sb[:], qscale[:])
            nc.scalar.dma_start(Wv_sb[:], wv[:])
            nc.vector.memset(ones_row[:], 1.0)
            nc.vector.memset(ones128[:], 1.0)
            nc.vector.memset(ones_blk[:], 1.0)
            nc.vector.tensor_copy(Vt[:, :, :, DH : DH + 1], ones_blk[:])
            # pin the single ACT table set (natural_log_exp_and_others):
            # every ACT fn used below (Ln, Exp, Square, Copy) is in it.
            nc.scalar.activation(dummy[:], ones_row[:, 0:2], AF.Ln)

            # ---- phase A: q,k projections (transposed layout) + raw sum-sq,
            # dc-outer with 4 parallel accumulation chains per half. The last
            # two dc steps of each chain are staggered so chain k's Square
            # (ACT) overlaps chains k+1.. matmuls and the half's stats are
            # ready ~3us after its last matmul instead of ~5us.
            for half in range(2):
                tiles = []
                for k in range(4):
                    pool, tag = (psp, "mm2") if k < 2 else (pav, "av")
                    tiles.append(
                        pool.tile([P, 2, 512], F32, tag=tag, name=f"a_{half}_{k}")
                    )
                for dc in range(DC - 2):
                    for k in range(4):
                        ct = 4 * half + k
                        for st in range(2):
                            nc.tensor.matmul(
                                tiles[k][:, st, :],
                                lhsT=Wqk_sb[:, ct, dc, :],
                                rhs=xT[:, dc, st * 512 : (st + 1) * 512],
                                start=(dc == 0),
                                stop=False,
                            )
                for k in range(4):
                    ct = 4 * half + k
                    for dc in (DC - 2, DC - 1):
                        for st in range(2):
                            nc.tensor.matmul(
                                tiles[k][:, st, :],
                                lhsT=Wqk_sb[:, ct, dc, :],
                                rhs=xT[:, dc, st * 512 : (st + 1) * 512],
                                start=False,
                                stop=(dc == DC - 1),
                            )
                    ps = tiles[k]
                    scr = scrp.tile([P, 2, 512], BF16, tag="scr")
                    nc.scalar.activation(
                        scr[:], ps[:], AF.Square, accum_out=sq_acc[:, ct : ct + 1]
                    )
                    flat = ps[:].rearrange("p a b -> p (a b)")
                    if ct < 4:
                        nc.vector.tensor_scalar(
                            QT[:, ct, :], flat, qs_sb[:, ct : ct + 1], None, ALU.mult
                        )
                    else:
                        nc.vector.tensor_copy(KT[:, ct - 4, :], flat)
                # per-half global-RMS partials: free-dim reduce on DVE, then
                # cross-partition reduce, then a [128,1]-replicated AllReduce
                # whose output DMAs straight into per-partition SBUF.
                hs = slice(4 * half, 4 * half + 4)
                nc.vector.reduce_sum(
                    qk2[:, half : half + 1], sq_acc[:, hs], axis=AX.X
                )
                if half == 0:
                    # q half: gpsimd all-reduce (PE busy with k chains; this
                    # collective's latency hides under the k half anyway).
                    nc.gpsimd.partition_all_reduce(
                        qred[:, 0:1], qk2[:, 0:1], 128, bass_isa.ReduceOp.add
                    )
                    nc.sync.dma_start(cc_ins[0][:], qred[:, 0:1])
                else:
                    # k half (critical): PE ones-matmul gives every partition
                    # the cross-partition sum in one tiny f32 matmul.
                    red_ps = pav.tile([P, 2, 512], F32, tag="av", name="red_ps")
                    nc.tensor.matmul(
                        red_ps[:, 0, 0:1],
                        lhsT=ones128[:],
                        rhs=qk2[:, 1:2],
                        start=True,
                        stop=True,
                    )
                    nc.vector.tensor_copy(qred[:, 1:2], red_ps[:, 0, 0:1])
                    nc.sync.dma_start(cc_ins[1][:], qred[:, 1:2])
                nc.gpsimd.collective_compute(
                    "AllReduce",
                    ALU.add,
                    replica_groups=REPLICAS,
                    ins=[cc_ins[half][:]],
                    outs=[cc_outs[half][:]],
                )
                nc.sync.dma_start(
                    gsum_bc[:, half : half + 1], cc_outs[half][:]
                )

            # ---- S-tile helpers (phase C) ----
            def s_mm(ps, h, skt):
                g, hp = h // 2, (h % 2) * DH
                for jj in range(2):
                    nc.tensor.matmul(
                        ps[:, jj, :],
                        lhsT=KT[hp : hp + DH, g, skt * P : (skt + 1) * P],
                        rhs=QT[hp : hp + DH, g, jj * 512 : (jj + 1) * 512],
                        start=True,
                        stop=True,
                    )

            def av_mm(ps_av, h, skc, E_t):
                for jj in range(2):
                    nc.tensor.matmul(
                        ps_av[0 : DH + 1, jj, :],
                        lhsT=Vt[:, skc, h, :],
                        rhs=E_t[:, skc, jj * 512 : (jj + 1) * 512],
                        start=(skc == 0),
                        stop=(skc == 7),
                    )

            def z_scale(ps_av, h, fast=False):
                # zT chunk g: low partitions = head 2g, high = head 2g+1,
                # EXCEPT chunk 3 which is swapped so the last-computed head
                # (3,1) takes the direct (no-DMA) path. Host wo layout matches.
                g, i = h // 2, h % 2
                direct = (i == 1) if g == 3 else (i == 0)
                av_flat = ps_av[0:DH, :, :].rearrange("p a b -> p (a b)")
                if fast:
                    # tail fast path: per-jj halves; DVE copies+recips for
                    # both halves first, then gpsimd broadcasts, then muls,
                    # so the gpsimd latency overlaps DVE work.
                    assert direct
                    rrhs, bchs = [], []
                    for jj in range(2):
                        rsh = smallp.tile(
                            [1, 512], F32, tag="rs0", name=f"rsh_{jj}"
                        )
                        nc.vector.tensor_copy(rsh[:], ps_av[DH : DH + 1, jj, :])
                        rrh = smallp.tile([1, 512], F32, tag="rs", name=f"rrh_{jj}")
                        nc.vector.reciprocal_approx_fast(rrh[:], rsh[:])
                        rrhs.append(rrh)
                    for jj in range(2):
                        bch = smallp.tile(
                            [DH, 512], F32, tag="bcs", name=f"bch_{jj}"
                        )
                        nc.gpsimd.partition_broadcast(bch[:], rrhs[jj][:])
                        bchs.append(bch)
                    for jj in range(2):
                        sl = slice(jj * 512, (jj + 1) * 512)
                        nc.vector.tensor_mul(
                            zT[0:DH, g, sl], ps_av[0:DH, jj, :], bchs[jj][:]
                        )
                    return
                rs0 = smallp.tile([1, S], F32, tag="rs0", name=f"rs0_{h}")
                nc.vector.tensor_copy(
                    rs0[:], ps_av[DH : DH + 1, :, :].rearrange("p a b -> p (a b)")
                )
                rs_r = smallp.tile([1, S], F32, tag="rs", name=f"rs_{h}")
                nc.vector.reciprocal_approx_fast(rs_r[:], rs0[:])
                bc_sb = smallp.tile([DH, S], F32, tag="bcs", name=f"bc_{h}")
                nc.gpsimd.partition_broadcast(bc_sb[:], rs_r[:])
                if direct:
                    nc.vector.tensor_mul(zT[0:DH, g, :], av_flat, bc_sb[:])
                else:
                    ztmp = smallp.tile([DH, S], BF16, tag="ztmp", name=f"zt_{h}")
                    nc.vector.tensor_mul(ztmp[:], av_flat, bc_sb[:])
                    nc.sync.dma_start(zT[DH:P, g, :], ztmp[:])

            # A2 (v projection) step generator: PE filler for the staging
            # window. Each chain sm: 8 accumulating matmuls into a pav tile,
            # then a DVE copy into Vt.
            a2_state = {"tile": None}

            def a2_step(step):
                sm, dc = step // 9, step % 9
                if dc == 8:
                    nc.vector.tensor_copy(
                        Vt[:, sm, :, 0:DH],
                        a2_state["tile"][:, 0, :].rearrange(
                            "p (h d) -> p h d", h=NHL
                        ),
                    )
                    a2_state["tile"] = None
                    return
                if dc == 0:
                    a2_state["tile"] = pav.tile(
                        [P, 2, 512], F32, tag="av", name=f"a2_{sm}"
                    )
                nc.tensor.matmul(
                    a2_state["tile"][:, 0, :],
                    lhsT=xT[:, dc, sm * P : (sm + 1) * P],
                    rhs=Wv_sb[:, dc, :],
                    start=(dc == 0),
                    stop=(dc == 7),
                )

            A2_STEPS = 8 * 9
            a2_done = 0

            nc.scalar.dma_start(Wo_sb[:], wo[:])

            # ---- staging window: pre-compute NSTG S tiles into the SBUF
            # stage ring while the collective is in flight; interleave the
            # A2 chains as PE filler so the PE stream is gapless.
            s_tiles = {}

            def emit_s_tile(t):
                ps = psp.tile([P, 2, 512], F32, tag="mm2", name=f"s_{t}")
                s_mm(ps, t // 8, t % 8)
                return ps

            a2_per_j = (A2_STEPS - 18 + NSTG - 1) // NSTG  # chains 0..5-ish
            for j in range(NSTG):
                ps = emit_s_tile(j)
                nc.vector.tensor_copy(
                    Sstage[:, j, :], ps[:].rearrange("p a b -> p (a b)")
                )
                tgt = min(A2_STEPS - 18, a2_done + a2_per_j)
                while a2_done < tgt:
                    a2_step(a2_done)
                    a2_done += 1
            # two extra S tiles into the freed psum slots + remaining A2
            s_tiles[NSTG] = emit_s_tile(NSTG)
            s_tiles[NSTG + 1] = emit_s_tile(NSTG + 1)
            while a2_done < A2_STEPS:
                a2_step(a2_done)
                a2_done += 1

            # ---- global scale: c = exp(-0.5 * ln(gq*gk / COUNT^2)),
            # computed per-partition (gsum_bc is [128,1]-replicated).
            nc.vector.tensor_mul(prod[:], gsum_bc[:, 0:1], gsum_bc[:, 1:2])
            nc.scalar.activation(lnp[:], prod[:], AF.Ln, scale=1.0 / COUNT**2)
            nc.scalar.activation(c_bc[:], lnp[:], AF.Exp, scale=-0.5)

            # ---- phase C: gapless exp stream on ACT; PE trails with S
            # production (psum-slot gated) and AV matmuls (lag NSTG -> 2).
            E = {}

            def get_E(h):
                if h not in E:
                    E[h] = ep.tile([P, 8, S], BF16, tag="E", name=f"E_{h}")
                return E[h]

            def emit_exp(t):
                E_t = get_E(t // 8)
                if t < NSTG:
                    nc.scalar.activation(
                        E_t[:, t % 8, :], Sstage[:, t, :], AF.Exp,
                        scale=c_bc[:, 0:1],
                    )
                else:
                    nc.scalar.activation(
                        E_t[:, t % 8, :], s_tiles.pop(t)[:], AF.Exp,
                        scale=c_bc[:, 0:1],
                    )

            av_ps = {}

            def emit_av(t):
                h, skc = t // 8, t % 8
                if skc == 0:
                    av_ps[h] = pav.tile(
                        [P, 2, 512], F32, tag="av", name=f"av_{h}"
                    )
                av_mm(av_ps[h], h, skc, get_E(h))
                if skc == 7:
                    z_scale(av_ps.pop(h), h, fast=(h == 7))

            for t in range(NSTG + 2):
                emit_exp(t)
            av_next = 0
            for j in range(NSTG + 2, 64):
                s_tiles[j] = emit_s_tile(j)
                emit_exp(j)
                allowed = min(j - 2, 2 * (j - (NSTG + 2)))
                while av_next <= allowed:
                    emit_av(av_next)
                    av_next += 1
            while av_next < 64:
                emit_av(av_next)
                av_next += 1

            # ---- phase D: output projection at the tail, two 4-dout waves,
            # Wo-stationary; copies alternate DVE / ACT.
            for wave in range(2):
                douts = list(range(4 * wave, 4 * wave + 4))
                tiles = {}
                for do_ in douts:
                    pool, tag = (psp, "mm2") if do_ % 4 < 2 else (pav, "av")
                    tiles[do_] = pool.tile(
                        [P, 2, 512], F32, tag=tag, name=f"o_{do_}"
                    )
                for gg in range(4):
                    for do_ in douts:
                        for nt in range(2):
                            nc.tensor.matmul(
                                tiles[do_][:, nt, :],
                                lhsT=Wo_sb[:, gg, do_ * P : (do_ + 1) * P],
                                rhs=zT[:, gg, nt * 512 : (nt + 1) * 512],
                                start=(gg == 0),
                                stop=(gg == 3),
                            )
                for do_ in douts:
                    ob = obp.tile([P, 2, 512], F32, tag="ob", name=f"ob_{do_}")
                    if do_ % 2:
                        nc.vector.tensor_copy(ob[:], tiles[do_][:])
                    else:
                        nc.scalar.activation(ob[:], tiles[do_][:], AF.Copy)
                    nc.sync.dma_start(
                        zpartT[do_ * P : (do_ + 1) * P, :],
                        ob[:].rearrange("p a b -> p (a b)"),
                    )

    nc.compile()
    return nc


def _get_nc():
    if "nc" not in _CACHE:
        _CACHE["nc"] = _build()
    return _CACHE["nc"]


def _prep_core_inputs(x, Wqkv, Wo, scale_q, scale_k):
    """Host-side shard + layout prep. Returns list of 8 in_maps."""
    x = np.asarray(x, dtype=np.float32)
    Wqkv = np.asarray(Wqkv, dtype=np.float32)
    Wo = np.asarray(Wo, dtype=np.float32)
    scale_q = np.asarray(scale_q, dtype=np.float32)
    scale_k = np.asarray(scale_k, dtype=np.float32)

    # combined per-d_head scale folded into Q (applied at the psum->SBUF copy)
    qs_vec = np.tile(scale_q * scale_k, NHL)               # [512]
    qs_dev = np.ascontiguousarray(qs_vec.reshape(4, P).T)  # [128,4]

    xt_all = []
    for b in range(4):
        xTb = x[b].T                                       # [d, s]
        xt_all.append(
            np.ascontiguousarray(
                xTb.reshape(DC, P, S).transpose(1, 0, 2).astype(NPBF)
            )
        )  # [128, 8, 1024]

    in_maps = []
    for c in range(8):
        b = c // 2
        hh = (c % 2) * NHL
        cols = slice(hh * DH, (hh + NHL) * DH)
        wq_c = Wqkv[:, 0 * D:1 * D][:, cols]               # [1024, 512]
        wk_c = Wqkv[:, 1 * D:2 * D][:, cols]
        wv_c = Wqkv[:, 2 * D:3 * D][:, cols]
        wqk_c = np.concatenate([wq_c, wk_c], axis=1)       # [1024, 1024]
        # [p, ct, dc, n]: per-ct slices are contiguous per-partition DMAs
        wqk_dev = np.ascontiguousarray(
            wqk_c.reshape(DC, P, 8, P).transpose(1, 2, 0, 3).astype(NPBF)
        )
        wv_dev = np.ascontiguousarray(
            wv_c.reshape(DC, P, NHL * DH).transpose(1, 0, 2).astype(NPBF)
        )
        # Wo rows for local heads, arranged [128, 4, 1024]:
        # chunk g low half = local head 2g, high half = local head 2g+1
        # (matches the zT packing of head pairs on partition halves)
        wo_loc = Wo[(hh * DH):(hh + NHL) * DH, :]          # [512, 1024]
        wo_dev = np.empty((P, 4, D), dtype=np.float32)
        for g in range(4):
            lo, hi = 2 * g, 2 * g + 1
            if g == 3:
                lo, hi = hi, lo  # chunk 3 head order swapped (see z_scale)
            wo_dev[0:DH, g, :] = wo_loc[lo * DH:(lo + 1) * DH, :]
            wo_dev[DH:P, g, :] = wo_loc[hi * DH:(hi + 1) * DH, :]
        in_maps.append(
            {
                "xt": xt_all[b],
                "wqk": wqk_dev,
                "wv": wv_dev,
                "wo": np.ascontiguousarray(wo_dev.astype(NPBF)),
                "qscale": qs_dev,
            }
        )
    return in_maps


def run(x, Wqkv, Wo, scale_q, scale_k, trace=False):
    nc = _get_nc()
    in_maps = _prep_core_inputs(x, Wqkv, Wo, scale_q, scale_k)
    res = run_bass_kernel_spmd(
        nc, in_maps, core_ids=list(range(N_CORES)), trace=trace
    )
    out = np.empty((4, S, D), dtype=np.float32)
    for b in range(4):
        zt = res.results[2 * b]["zpartT"] + res.results[2 * b + 1]["zpartT"]
        out[b] = zt.T
    return out, res


def kernel(x, Wqkv, Wo, scale_q, scale_k):
    out, _ = run(x, Wqkv, Wo, scale_q, scale_k, trace=False)
    return out


# revision 7
# speedup vs baseline: 1.1938x; 1.1938x over previous
"""TRN2 Bass kernel for nn_Attention_188978561266.

Reference computation (b=4, s=1024, d=1024, 16 heads x 64):
    qkv = x @ Wqkv ; split q,k,v
    q = q / (sqrt(mean(q^2 over ALL elements)) + eps) * scale_q   (global scalar RMS)
    k = k / (sqrt(mean(k^2 over ALL elements)) + eps) * scale_k
    attn = softmax(q @ k^T)  (no 1/sqrt(d_head), no mask)
    out = (attn @ v) @ Wo

Sharding: 8 cores = (batch b in 0..3) x (head-half in 0..1). Each core computes
qkv for its batch restricted to its 8 heads (tensor-parallel QKV columns),
full attention for those heads, and a partial (transposed) output projection.
Host sums the two partial outputs per batch and transposes. The global RMS
needs a cross-core AllReduce of the (sum q^2, sum k^2) scalars.

v2 design notes (measured on HW via neuron-profile traces):
  - Phase C is ACT-exp-bound: 64 exp tiles of [128,1024] at ~1.15us each.
    v1 ran PE and ACT in per-tile lockstep; the PE's micro-idles kept the
    HAM clock gate at K=4/8 (1.2GHz) where the PE cannot feed the exp
    stream, locking a ~1.95us/tile slow state. v2 decouples them:
      * During the collective wait the PE pre-computes NSTG S tiles whose
        psums the DVE copies into an SBUF stage ring; the ring is an
        ~11us exp-credit buffer absorbing any PE clock transient.
      * Steady state exps read S psums directly (DVE psum-copies are 1x
        mode and cannot keep up as an intermediary).
      * The V projection (A2) is interleaved into the staging window as
        real PE filler so the PE stream has no idle gaps there.
      * AV matmuls trail the exp stream with a lag that starts at NSTG
        and shrinks to 2, so the tail drain is minimal.
  - One ACT table set for the whole kernel (natural_log_exp_and_others):
    the global scale is c = exp(-0.5*ln(mq*mk)) (no Sqrt table), Square
    and Copy are fillers present in every set, and a dummy Ln at t0 pins
    the set. No table switch ever lands on the critical path.
  - The stats partition-reduction for the critical k half is a PE
    ones-matmul (f32): out[128,1] = ones128.T @ qk2 gives every partition
    the cross-partition sum in ~0.2us (vs ~1.3us gpsimd ucode). The q
    half (off critical path, PE busy) stays on gpsimd.
  - Collectives carry [128,1] replicated payloads so the AllReduce result
    DMAs straight into per-partition SBUF: no partition_broadcast after
    the collective. c_bc = exp(-0.5 ln(gq*gk/COUNT^2)) is computed
    per-partition in ~0.5us (DVE mult + 2 tiny ACT ops).
  - The warm dummy collective (absorbs ~35us CC cold start) and the split
    q-half/k-half collectives (q absorbs cross-core skew so k runs warm)
    are retained from v1.
  - Output projection at the tail, Wo-stationary, two 4-dout waves; zT
    chunk 3 is head-swapped so the last head takes the no-DMA path;
    psum->SBUF copies alternate DVE / ACT (ACT is free at the tail).
  - PSUM budget: 2x psp + 2x pav tiles of [128,2,512] = 8 banks.
"""

import os as _os
import sys

sys.path.insert(0, "/opt/trn_rl_repo")

import ml_dtypes
import numpy as np

import concourse.bacc as bacc
import concourse.bass as bass
import concourse.mybir as mybir
from concourse import bass_isa, library_config, tile
from concourse.bass_utils import run_bass_kernel_spmd

F32 = mybir.dt.float32
I32 = mybir.dt.int32
F32R = mybir.dt.float32r
BF16 = mybir.dt.bfloat16
AF = mybir.ActivationFunctionType
ALU = mybir.AluOpType
AX = mybir.AxisListType

NPBF = ml_dtypes.bfloat16

P = 128
D = 1024
S = 1024
N_HEAD = 16
DH = 64
NHL = 8          # heads per core
DC = 8           # d contraction chunks of 128
EPS = 1e-6
COUNT = 4 * 1024 * 1024   # elements of the full q (or k) tensor
N_CORES = 8
REPLICAS = [list(range(N_CORES))]

NSTG = int(_os.environ.get("KN_NSTG", "9"))   # staged S tiles (exp credit)

_CACHE = {}


def _build():
    nc = bacc.Bacc("TRN2", target_bir_lowering=False, debug=False, num_devices=N_CORES)

    xt = nc.dram_tensor("xt", [P, DC, S], BF16, kind="ExternalInput")
    wqk = nc.dram_tensor("wqk", [P, DC, 8, P], BF16, kind="ExternalInput")
    wv = nc.dram_tensor("wv", [P, DC, NHL * DH], BF16, kind="ExternalInput")
    wo = nc.dram_tensor("wo", [P, 4, D], BF16, kind="ExternalInput")
    qscale = nc.dram_tensor("qscale", [P, 4], F32, kind="ExternalInput")
    zpartT = nc.dram_tensor("zpartT", [D, S], F32, kind="ExternalOutput")

    with tile.TileContext(nc) as tc:
        with (
            tc.tile_pool(name="big", bufs=1) as big,
            tc.tile_pool(name="ep", bufs=3) as ep,
            tc.tile_pool(name="scr", bufs=2) as scrp,
            tc.tile_pool(name="ob", bufs=2) as obp,
            tc.tile_pool(name="small", bufs=2) as smallp,
            tc.tile_pool(name="stats", bufs=1) as stp,
            tc.tile_pool(name="ps", bufs=2, space="PSUM") as psp,
            tc.tile_pool(name="pav", bufs=2, space="PSUM") as pav,
            tc.tile_pool(name="dram", bufs=1, space="DRAM") as dramp,
        ):
            # ---- persistent SBUF tensors ----
            xT = big.tile([P, DC, S], BF16, tag="xT")
            Wqk_sb = big.tile([P, DC, 8, P], BF16, tag="Wqk")
            QT = big.tile([P, 4, S], BF16, tag="QT")
            KT = big.tile([P, 4, S], BF16, tag="KT")
            Vt = big.tile([P, 8, NHL, DH + 1], BF16, tag="Vt")
            zT = big.tile([P, 4, S], BF16, tag="zT")
            Wo_sb = big.tile([P, 4, D], BF16, tag="Wo")
            Wv_sb = big.tile([P, DC, NHL * DH], BF16, tag="Wv")

            Sstage = big.tile([P, NSTG, S], F32, tag="Sstage")
            ones128 = big.tile([P, P], F32, tag="ones128")
            qs_sb = stp.tile([P, 4], F32, tag="qs")
            sq_acc = stp.tile([P, 8], F32, tag="sqacc")
            qk2 = stp.tile([P, 2], F32, tag="qk2")
            qred = stp.tile([P, 2], F32, tag="qred")
            gsum_bc = stp.tile([P, 2], F32, tag="gsum")
            prod = stp.tile([P, 1], F32, tag="prod")
            cby = stp.tile([P, 1], F32, tag="cby")
            cbt = stp.tile([P, 1], F32, tag="cbt")
            magic = stp.tile([P, 1], I32, tag="magic")
            c_bc = stp.tile([P, 1], F32, tag="cbc")

            ones_row = stp.tile([1, P], F32, tag="ones_row")
            ones_blk = stp.tile([P, 8, NHL, 1], F32, tag="ones_blk")

            # warm collective first: starts the CC firmware warmup / skew
            # barrier as early as possible (it does not need the library).
            cc_warm_in = dramp.tile([2, 1], F32, tag="ccwi")
            cc_warm_out = dramp.tile([2, 1], F32, tag="ccwo", addr_space="Shared")
            cc_in = dramp.tile([P, 2], F32, tag="cci", name="cc_in")
            cc_out = dramp.tile(
                [P, 2], F32, tag="cco", addr_space="Shared", name="cc_out"
            )
            if _os.environ.get("KN_WARMCC", "1") == "1":
                nc.gpsimd.collective_compute(
                    "AllReduce",
                    ALU.add,
                    replica_groups=REPLICAS,
                    ins=[cc_warm_in[:]],
                    outs=[cc_warm_out[:]],
                )
            nc.gpsimd.load_library(library_config.attn)

            # ---- input DMAs + consts; wqk cts 0-3 then x chunks in arrival
            # order on two queues so the dc-outer phase A starts earliest.
            for lo, hi in ((0, 1), (1, 4), (4, 6), (6, 8)):
                nc.sync.dma_start(xT[:, lo:hi, :], xt[:, lo:hi, :])
                nc.scalar.dma_start(
                    Wqk_sb[:, lo:hi, :, :], wqk[:, lo:hi, :, :]
                )
            nc.sync.dma_start(qs_sb[:], qscale[:])
            nc.scalar.dma_start(Wv_sb[:], wv[:])
            nc.vector.memset(ones_row[:], 1.0)
            nc.vector.memset(ones128[:], 1.0)
            nc.vector.memset(ones_blk[:], 1.0)
            nc.vector.tensor_copy(Vt[:, :, :, DH : DH + 1], ones_blk[:])
            nc.vector.memset(magic[:], 0x5F3759DF)

            # ---- phase A: q,k projections (transposed layout) + raw sum-sq,
            # dc-outer with 4 parallel accumulation chains per half. The last
            # two dc steps of each chain are staggered so chain k's Square
            # (ACT) overlaps chains k+1.. matmuls and the half's stats are
            # ready ~3us after its last matmul instead of ~5us.
            for half in range(2):
                tiles = []
                for k in range(4):
                    pool, tag = (psp, "mm2") if k < 2 else (pav, "av")
                    tiles.append(
                        pool.tile([P, 2, 512], F32, tag=tag, name=f"a_{half}_{k}")
                    )
                for dc in range(DC - 2):
                    for k in range(4):
                        ct = 4 * half + k
                        for st in range(2):
                            nc.tensor.matmul(
                                tiles[k][:, st, :],
                                lhsT=Wqk_sb[:, dc, ct, :],
                                rhs=xT[:, dc, st * 512 : (st + 1) * 512],
                                start=(dc == 0),
                                stop=False,
                            )
                for k in range(4):
                    ct = 4 * half + k
                    for dc in (DC - 2, DC - 1):
                        for st in range(2):
                            nc.tensor.matmul(
                                tiles[k][:, st, :],
                                lhsT=Wqk_sb[:, dc, ct, :],
                                rhs=xT[:, dc, st * 512 : (st + 1) * 512],
                                start=False,
                                stop=(dc == DC - 1),
                            )
                    ps = tiles[k]
                    scr = scrp.tile([P, 2, 512], BF16, tag="scr")
                    nc.scalar.activation(
                        scr[:], ps[:], AF.Square, accum_out=sq_acc[:, ct : ct + 1]
                    )
                    flat = ps[:].rearrange("p a b -> p (a b)")
                    if ct < 4:
                        nc.vector.tensor_scalar(
                            QT[:, ct, :], flat, qs_sb[:, ct : ct + 1], None, ALU.mult
                        )
                    else:
                        nc.vector.tensor_copy(KT[:, ct - 4, :], flat)
                # per-half global-RMS partials: free-dim reduce on DVE, then
                # cross-partition reduce, then a [128,1]-replicated AllReduce
                # whose output DMAs straight into per-partition SBUF.
                hs = slice(4 * half, 4 * half + 4)
                nc.vector.reduce_sum(
                    qk2[:, half : half + 1], sq_acc[:, hs], axis=AX.X
                )
            # single combined collective right after the k stats: the warm
            # collective at t0 already absorbed CC cold start + core skew,
            # so splitting q/k only serializes on the CC cores. The PE
            # ones-matmul gives every partition both cross-partition sums.
            red_ps = pav.tile([P, 2, 512], F32, tag="av", name="red_ps")
            nc.tensor.matmul(
                red_ps[:, 0, 0:2],
                lhsT=ones128[:],
                rhs=qk2[:, 0:2],
                start=True,
                stop=True,
            )
            nc.vector.tensor_copy(qred[:, 0:2], red_ps[:, 0, 0:2])
            nc.sync.dma_start(cc_in[:], qred[:, 0:2])
            nc.gpsimd.collective_compute(
                "AllReduce",
                ALU.add,
                replica_groups=REPLICAS,
                ins=[cc_in[:]],
                outs=[cc_out[:]],
            )
            nc.sync.dma_start(gsum_bc[:, 0:2], cc_out[:])

            # ---- S-tile helpers (phase C) ----
            def s_mm(ps, h, skt):
                g, hp = h // 2, (h % 2) * DH
                for jj in range(2):
                    nc.tensor.matmul(
                        ps[:, jj, :],
                        lhsT=KT[hp : hp + DH, g, skt * P : (skt + 1) * P],
                        rhs=QT[hp : hp + DH, g, jj * 512 : (jj + 1) * 512],
                        start=True,
                        stop=True,
                    )

            def av_mm(ps_av, h, skc, E_t):
                for jj in range(2):
                    nc.tensor.matmul(
                        ps_av[0 : DH + 1, jj, :],
                        lhsT=Vt[:, skc, h, :],
                        rhs=E_t[:, skc, jj * 512 : (jj + 1) * 512],
                        start=(skc == 0),
                        stop=(skc == 7),
                    )

            def z_scale(ps_av, h, fast=False):
                # zT chunk g: low partitions = head 2g, high = head 2g+1,
                # EXCEPT chunk 3 which is swapped so the last-computed head
                # (3,1) takes the direct (no-DMA) path. Host wo layout matches.
                g, i = h // 2, h % 2
                direct = (i == 1) if g == 3 else (i == 0)
                av_flat = ps_av[0:DH, :, :].rearrange("p a b -> p (a b)")
                if fast:
                    # tail fast path: per-jj halves; DVE copies+recips for
                    # both halves first, then gpsimd broadcasts, then muls,
                    # so the gpsimd latency overlaps DVE work.
                    assert direct
                    rrhs, bchs = [], []
                    for jj in range(2):
                        rsh = smallp.tile(
                            [1, 512], F32, tag="rs0", name=f"rsh_{jj}"
                        )
                        nc.vector.tensor_copy(rsh[:], ps_av[DH : DH + 1, jj, :])
                        rrh = smallp.tile([1, 512], F32, tag="rs", name=f"rrh_{jj}")
                        nc.vector.reciprocal_approx_fast(rrh[:], rsh[:])
                        rrhs.append(rrh)
                    for jj in range(2):
                        bch = smallp.tile(
                            [DH, 512], F32, tag="bcs", name=f"bch_{jj}"
                        )
                        nc.gpsimd.partition_broadcast(bch[:], rrhs[jj][:])
                        bchs.append(bch)
                    for jj in range(2):
                        sl = slice(jj * 512, (jj + 1) * 512)
                        nc.vector.tensor_mul(
                            zT[0:DH, g, sl], ps_av[0:DH, jj, :], bchs[jj][:]
                        )
                    return
                rs0 = smallp.tile([1, S], F32, tag="rs0", name=f"rs0_{h}")
                nc.vector.tensor_copy(
                    rs0[:], ps_av[DH : DH + 1, :, :].rearrange("p a b -> p (a b)")
                )
                rs_r = smallp.tile([1, S], F32, tag="rs", name=f"rs_{h}")
                nc.vector.reciprocal_approx_fast(rs_r[:], rs0[:])
                bc_sb = smallp.tile([DH, S], F32, tag="bcs", name=f"bc_{h}")
                nc.gpsimd.partition_broadcast(bc_sb[:], rs_r[:])
                if direct:
                    nc.vector.tensor_mul(zT[0:DH, g, :], av_flat, bc_sb[:])
                else:
                    ztmp = smallp.tile([DH, S], BF16, tag="ztmp", name=f"zt_{h}")
                    nc.vector.tensor_mul(ztmp[:], av_flat, bc_sb[:])
                    nc.sync.dma_start(zT[DH:P, g, :], ztmp[:])

            # A2 (v projection) step generator: PE filler for the staging
            # window. Each chain sm: 8 accumulating matmuls into a pav tile,
            # then a DVE copy into Vt.
            a2_state = {"tile": None}

            def a2_step(step):
                sm, dc = step // 9, step % 9
                if dc == 8:
                    nc.vector.tensor_copy(
                        Vt[:, sm, :, 0:DH],
                        a2_state["tile"][:, 0, :].rearrange(
                            "p (h d) -> p h d", h=NHL
                        ),
                    )
                    a2_state["tile"] = None
                    return
                if dc == 0:
                    a2_state["tile"] = pav.tile(
                        [P, 2, 512], F32, tag="av", name=f"a2_{sm}"
                    )
                nc.tensor.matmul(
                    a2_state["tile"][:, 0, :],
                    lhsT=xT[:, dc, sm * P : (sm + 1) * P],
                    rhs=Wv_sb[:, dc, :],
                    start=(dc == 0),
                    stop=(dc == 7),
                )

            A2_STEPS = 8 * 9
            a2_done = 0

            nc.scalar.dma_start(Wo_sb[:], wo[:])

            # ---- staging window: pre-compute NSTG S tiles into the SBUF
            # stage ring while the collective is in flight; interleave the
            # A2 chains as PE filler so the PE stream is gapless.
            s_tiles = {}

            def emit_s_tile(t):
                ps = psp.tile([P, 2, 512], F32, tag="mm2", name=f"s_{t}")
                s_mm(ps, t // 8, t % 8)
                return ps

            a2_per_j = (A2_STEPS - 18 + NSTG - 1) // NSTG  # chains 0..5-ish
            for j in range(NSTG):
                ps = emit_s_tile(j)
                nc.vector.tensor_copy(
                    Sstage[:, j, :], ps[:].rearrange("p a b -> p (a b)")
                )
                tgt = min(A2_STEPS - 18, a2_done + a2_per_j)
                while a2_done < tgt:
                    a2_step(a2_done)
                    a2_done += 1
            # two extra S tiles into the freed psum slots + remaining A2
            s_tiles[NSTG] = emit_s_tile(NSTG)
            s_tiles[NSTG + 1] = emit_s_tile(NSTG + 1)
            while a2_done < A2_STEPS:
                a2_step(a2_done)
                a2_done += 1

            # ---- global scale: c = COUNT / sqrt(gq*gk), computed
            # per-partition on DVE only (quake rsqrt seed + 2 Newton steps,
            # ~5e-6 rel err): no ACT table traffic on the critical path.
            nc.vector.tensor_mul(prod[:], gsum_bc[:, 0:1], gsum_bc[:, 1:2])
            nc.vector.tensor_scalar(
                cby[:].bitcast(I32), prod[:].bitcast(I32), 1, None,
                ALU.logical_shift_right,
            )
            nc.vector.tensor_tensor(
                cby[:].bitcast(I32), magic[:], cby[:].bitcast(I32), ALU.subtract
            )
            for _ in range(2):
                nc.vector.tensor_mul(cbt[:], cby[:], cby[:])
                nc.vector.tensor_mul(cbt[:], cbt[:], prod[:])
                nc.vector.tensor_scalar(cbt[:], cbt[:], -0.5, 1.5, ALU.mult, ALU.add)
                nc.vector.tensor_mul(cby[:], cby[:], cbt[:])
            nc.vector.tensor_scalar_mul(c_bc[:], cby[:], float(COUNT))

            # ---- phase C: gapless exp stream on ACT; PE trails with S
            # production (psum-slot gated) and AV matmuls (lag NSTG -> 2).
            E = {}

            def get_E(h):
                if h not in E:
                    E[h] = ep.tile([P, 8, S], BF16, tag="E", name=f"E_{h}")
                return E[h]

            def emit_exp(t):
                E_t = get_E(t // 8)
                if t < NSTG:
                    nc.scalar.activation(
                        E_t[:, t % 8, :], Sstage[:, t, :], AF.Exp,
                        scale=c_bc[:, 0:1],
                    )
                else:
                    nc.scalar.activation(
                        E_t[:, t % 8, :], s_tiles.pop(t)[:], AF.Exp,
                        scale=c_bc[:, 0:1],
                    )

            av_ps = {}

            def emit_av(t):
                h, skc = t // 8, t % 8
                if skc == 0:
                    av_ps[h] = pav.tile(
                        [P, 2, 512], F32, tag="av", name=f"av_{h}"
                    )
                av_mm(av_ps[h], h, skc, get_E(h))
                if skc == 7:
                    z_scale(av_ps.pop(h), h, fast=(h == 7))

            for t in range(NSTG + 2):
                emit_exp(t)
            av_next = 0
            for j in range(NSTG + 2, 64):
                s_tiles[j] = emit_s_tile(j)
                emit_exp(j)
                allowed = min(j - 2, 2 * (j - (NSTG + 2)))
                while av_next <= allowed:
                    emit_av(av_next)
                    av_next += 1
            while av_next < 64:
                emit_av(av_next)
                av_next += 1

            # ---- phase D: output projection at the tail, two 4-dout waves,
            # Wo-stationary; copies alternate DVE / ACT.
            for wave in range(2):
                douts = list(range(4 * wave, 4 * wave + 4))
                tiles = {}
                for do_ in douts:
                    pool, tag = (psp, "mm2") if do_ % 4 < 2 else (pav, "av")
                    tiles[do_] = pool.tile(
                        [P, 2, 512], F32, tag=tag, name=f"o_{do_}"
                    )
                for gg in range(4):
                    for do_ in douts:
                        for nt in range(2):
                            nc.tensor.matmul(
                                tiles[do_][:, nt, :],
                                lhsT=Wo_sb[:, gg, do_ * P : (do_ + 1) * P],
                                rhs=zT[:, gg, nt * 512 : (nt + 1) * 512],
                                start=(gg == 0),
                                stop=(gg == 3),
                            )
                for do_ in douts:
                    ob = obp.tile([P, 2, 512], F32, tag="ob", name=f"ob_{do_}")
                    if do_ % 2:
                        nc.vector.tensor_copy(ob[:], tiles[do_][:])
                    else:
                        nc.scalar.activation(ob[:], tiles[do_][:], AF.Copy)
                    nc.sync.dma_start(
                        zpartT[do_ * P : (do_ + 1) * P, :],
                        ob[:].rearrange("p a b -> p (a b)"),
                    )

    nc.compile()
    return nc


def _get_nc():
    if "nc" not in _CACHE:
        _CACHE["nc"] = _build()
    return _CACHE["nc"]


def _prep_core_inputs(x, Wqkv, Wo, scale_q, scale_k):
    """Host-side shard + layout prep. Returns list of 8 in_maps."""
    x = np.asarray(x, dtype=np.float32)
    Wqkv = np.asarray(Wqkv, dtype=np.float32)
    Wo = np.asarray(Wo, dtype=np.float32)
    scale_q = np.asarray(scale_q, dtype=np.float32)
    scale_k = np.asarray(scale_k, dtype=np.float32)

    # combined per-d_head scale folded into Q (applied at the psum->SBUF copy)
    qs_vec = np.tile(scale_q * scale_k, NHL)               # [512]
    qs_dev = np.ascontiguousarray(qs_vec.reshape(4, P).T)  # [128,4]

    xt_all = []
    for b in range(4):
        xTb = x[b].T                                       # [d, s]
        xt_all.append(
            np.ascontiguousarray(
                xTb.reshape(DC, P, S).transpose(1, 0, 2).astype(NPBF)
            )
        )  # [128, 8, 1024]

    in_maps = []
    for c in range(8):
        b = c // 2
        hh = (c % 2) * NHL
        cols = slice(hh * DH, (hh + NHL) * DH)
        wq_c = Wqkv[:, 0 * D:1 * D][:, cols]               # [1024, 512]
        wk_c = Wqkv[:, 1 * D:2 * D][:, cols]
        wv_c = Wqkv[:, 2 * D:3 * D][:, cols]
        wqk_c = np.concatenate([wq_c, wk_c], axis=1)       # [1024, 1024]
        # [p, ct, dc, n]: per-ct slices are contiguous per-partition DMAs
        # [p, dc, ct, n]: dc-major so the dc-outer phase A consumes the
        # chunks in DMA arrival order; per-dc slices are contiguous 2KB+
        # per-partition DMA lines.
        wqk_dev = np.ascontiguousarray(
            wqk_c.reshape(DC, P, 8, P).transpose(1, 0, 2, 3).astype(NPBF)
        )
        wv_dev = np.ascontiguousarray(
            wv_c.reshape(DC, P, NHL * DH).transpose(1, 0, 2).astype(NPBF)
        )
        # Wo rows for local heads, arranged [128, 4, 1024]:
        # chunk g low half = local head 2g, high half = local head 2g+1
        # (matches the zT packing of head pairs on partition halves)
        wo_loc = Wo[(hh * DH):(hh + NHL) * DH, :]          # [512, 1024]
        wo_dev = np.empty((P, 4, D), dtype=np.float32)
        for g in range(4):
            lo, hi = 2 * g, 2 * g + 1
            if g == 3:
                lo, hi = hi, lo  # chunk 3 head order swapped (see z_scale)
            wo_dev[0:DH, g, :] = wo_loc[lo * DH:(lo + 1) * DH, :]
            wo_dev[DH:P, g, :] = wo_loc[hi * DH:(hi + 1) * DH, :]
        in_maps.append(
            {
                "xt": xt_all[b],
                "wqk": wqk_dev,
                "wv": wv_dev,
                "wo": np.ascontiguousarray(wo_dev.astype(NPBF)),
                "qscale": qs_dev,
            }
        )
    return in_maps


def run(x, Wqkv, Wo, scale_q, scale_k, trace=False):
    nc = _get_nc()
    in_maps = _prep_core_inputs(x, Wqkv, Wo, scale_q, scale_k)
    res = run_bass_kernel_spmd(
        nc, in_maps, core_ids=list(range(N_CORES)), trace=trace
    )
    out = np.empty((4, S, D), dtype=np.float32)
    for b in range(4):
        zt = res.results[2 * b]["zpartT"] + res.results[2 * b + 1]["zpartT"]
        out[b] = zt.T
    return out, res


def kernel(x, Wqkv, Wo, scale_q, scale_k):
    out, _ = run(x, Wqkv, Wo, scale_q, scale_k, trace=False)
    return out
